# revision 25
# baseline (speedup 1.0000x reference)
"""Trainium2 Bass kernel for nn_BatchMatMulModule.

Computes out = einsum("bnij,bmj->bnmi", x, y) with
  x: [4, 64, 3, 3] f32, y: [4, 100000, 3] f32 -> out: [4, 64, 100000, 3] f32.

The output (307 MB) dwarfs the inputs (4.8 MB), so the kernel is bound by
HBM write bandwidth (~358 GB/s per NeuronCore => ~107 us floor for the
38.4 MB each core stores). Strategy (v2, TensorE-based):

- Shard the 256 flat (b, n) pairs across 8 cores: core c handles b = c // 2
  and 32 consecutive n. Output slice [32, 100000, 3] is contiguous in DRAM.
- SBUF/PSUM layout: partition p = a * 32 + s with a = n % 4 (within a group
  of 4 n's) and s = m-segment (100000 = 32 segments x 3125 rows).
- The contraction out[(a,s), t, i] = sum_j x[n,i,j] * y[(s,t),j] is run on
  the idle TensorE as a matmul with a block-diagonal stationary operand:
    W_{g,i}[(s',j), (a,s)] = x[g*4+a, i, j] * delta(s,s')   [96 x 128]
    Y[(s',j), t] = y[s'*3125 + t, j]                        [96 x 3125]
    psum_i = W_{g,i}.T @ Y[:, t0:t1]                        [128 x <=512]
  Operands are fp32 bitcast to float32r (FP22-truncated): full PE rate at
  N >= 256, rel err ~1e-4 (harness gate is 2e-2).
- PSUM cannot be DMA'd, so the mandatory PSUM->SBUF evacuation doubles as
  the i-interleave: one ACT/DVE copy per chunk reads the 3 psum planes
  (strided) and writes the (t, i)-interleaved SBUF tile (contiguous), which
  then stores as a 768 KB contiguous-per-partition DMA.
- Engine budget per core: DMA-out ~107 us (bottleneck), PE ~25 us,
  ACT/DVE alternate evacuation chunks at ~40 us each.
"""

import numpy as np

import concourse.bacc as bacc
import concourse.mybir as mybir
from concourse.bass_utils import run_bass_kernel_spmd
from concourse.tile import TileContext

N_CORES = 8
P = 128
N_PER_CORE = 32   # n per core
N_SUB = 4         # a: n's packed per partition-group
N_GROUPS = N_PER_CORE // N_SUB  # 8 groups, one weight set each
SEGS = P // N_SUB               # 32 m-segments
M = 100000
ROWS = M // SEGS                # 3125 t-rows per segment
K = SEGS * 3                    # 96 contraction rows (s', j)

CHUNK = 512                     # t-chunk = one PSUM bank of fp32
CHUNK_STARTS = list(range(0, ROWS, CHUNK))
ROWS_PAD = ROWS + (ROWS % 2)    # fp32r matmul needs even free sizes; pad tail

TRACE = False
LAST = None  # last BassKernelResults, for test harness introspection

_CACHED_NC = None

# Tuning knobs.
EVAC_MODE = "fused"  # "fused": 1 copy/chunk (strided psum read, contig write)
                     # "plane3": 3 copies/chunk (contig read, strided write)
OUT_BUFS = 5
TAIL_BUFS = 5   # bufs for the merged tail store group's tiles
PSUM_BUFS = 2
Y_SPLITS = 4
STORE_CHUNKS = 3     # chunks coalesced per store DMA (trailing short group merged)
STORE_ENGINE = "sync"  # "sync" | "alt" (alternate sync/scalar HWDGE queues)


def build_bass(reps: int = 1, ops_mode: str = "full"):
    nc = bacc.Bacc(
        "TRN2",
        debug=False,
        enable_asserts=False,
        target_bir_lowering=False,
        num_devices=N_CORES,
    )
    f32 = mybir.dt.float32
    f32r = mybir.dt.float32r
    copy = mybir.ActivationFunctionType.Copy

    # Host-prearranged inputs (float32r: fp32 bytes, FP22-truncated by the PE):
    #  xw[k, (g*3+i)*128 + a*32 + s] = x[g*4+a, i, j] * delta(s, k//3), j=k%3
    #  ys[k, t] = y[b, (k//3)*3125 + t, k%3]
    xw = nc.dram_tensor("xw", [K, N_GROUPS * 3 * P], f32r, kind="ExternalInput").ap()
    ys = nc.dram_tensor("ys", [K, ROWS_PAD], f32r, kind="ExternalInput").ap()
    out = nc.dram_tensor("out", [N_PER_CORE, M, 3], f32, kind="ExternalOutput").ap()

    with TileContext(nc) as tc:
        with (
            tc.tile_pool(name="const", bufs=1) as cpool,
            tc.tile_pool(name="outp", bufs=OUT_BUFS) as opool,
            tc.tile_pool(name="psp", bufs=PSUM_BUFS, space="PSUM") as pspool,
        ):
            wt = cpool.tile([K, N_GROUPS * 3 * P], f32r)
            # First group's weights first so g=0 matmuls start early.
            nc.sync.dma_start(out=wt[:, : 3 * P], in_=xw[:, : 3 * P])
            nc.sync.dma_start(out=wt[:, 3 * P:], in_=xw[:, 3 * P:])

            y_tile = cpool.tile([K, ROWS_PAD], f32r)
            yb = [ROWS_PAD * h // Y_SPLITS for h in range(Y_SPLITS + 1)]
            for h in range(Y_SPLITS):
                nc.sync.dma_start(
                    out=y_tile[:, yb[h]:yb[h + 1]], in_=ys[:, yb[h]:yb[h + 1]]
                )

            out_r = out.rearrange("(g a) (s t) i -> g a s t i", a=N_SUB, s=SEGS)

            gt = None
            if ops_mode in ("store", "noevac"):
                gt = cpool.tile([P, 3 * CHUNK], f32)
                nc.vector.memset(gt[:], 0.0)

            def emit_compute(g, c, u, ot, col0):
                """Matmuls + evacuation for chunk (g, c); interleaved result
                lands at ot[:, col0 : col0 + 3*nt]. Returns nt."""
                t0 = CHUNK_STARTS[c]
                nt = min(CHUNK, ROWS - t0)           # rows actually stored
                nt_mm = nt + (nt % 2)                # fp32r: even matmul width
                pst = pspool.tile([P, 3 * CHUNK], f32, name="ps", tag="ps")
                for i in range(3):
                    blk = (g * 3 + i) * P
                    nc.tensor.matmul(
                        pst[:, i * CHUNK: i * CHUNK + nt_mm],
                        wt[:, blk: blk + P],
                        y_tile[:, t0: t0 + nt_mm],
                        start=True,
                        stop=True,
                    )
                if ops_mode == "noevac":
                    return nt
                # psum viewed [p, i, t] (planes at CHUNK spacing), out (t, i).
                psv = pst.rearrange("p (i t) -> p i t", i=3)[:, :, :nt]
                otv = ot[:, col0: col0 + 3 * nt]
                if EVAC_MODE == "fused":
                    src = psv.rearrange("p i t -> p t i")
                    dst_sb = otv.rearrange("p (t i) -> p t i", i=3)
                    if u % 2 == 0:
                        nc.scalar.activation(out=dst_sb, in_=src, func=copy)
                    else:
                        nc.vector.tensor_copy(out=dst_sb, in_=src)
                else:
                    ov = otv.rearrange("p (t i) -> p t i", i=3)
                    for i in range(3):
                        if (i + u) % 3 != 0:
                            nc.scalar.activation(
                                out=ov[:, :, i], in_=psv[:, i, :], func=copy)
                        else:
                            nc.vector.tensor_copy(
                                out=ov[:, :, i], in_=psv[:, i, :])
                return nt

            store_n = [0]

            def emit_store(g, cs, ot, total_nt):
                t0 = CHUNK_STARTS[cs[0]]
                dst = out_r[g][:, :, t0: t0 + total_nt, :].rearrange(
                    "a s t i -> (a s) t i")
                eng = nc.sync
                if STORE_ENGINE == "alt" and store_n[0] % 2 == 1:
                    eng = nc.scalar
                store_n[0] += 1
                eng.dma_start(out=dst, in_=ot[:, : 3 * total_nt])

            nch = len(CHUNK_STARTS)
            groups = [list(range(c, min(c + STORE_CHUNKS, nch)))
                      for c in range(0, nch, STORE_CHUNKS)]
            if len(groups) > 1 and len(groups[-1]) == 1:
                groups[-2].extend(groups.pop())

            u = 0
            for _ in range(reps):
                for g in range(N_GROUPS):
                    for cs in groups:
                        span = 3 * (CHUNK * (len(cs) - 1)
                                    + min(CHUNK, ROWS - CHUNK_STARTS[cs[-1]]))
                        if ops_mode == "store":
                            # store-only probe: reuse the const garbage tile
                            for c in cs:
                                nt = min(CHUNK, ROWS - CHUNK_STARTS[c])
                                dst = out_r[g][
                                    :, :, CHUNK_STARTS[c]:
                                    CHUNK_STARTS[c] + nt, :].rearrange(
                                    "a s t i -> (a s) t i")
                                nc.sync.dma_start(
                                    out=dst, in_=gt[:, : 3 * nt])
                            u += len(cs)
                            continue
                        ot = opool.tile(
                            [P, span], f32, name="ot",
                            tag=f"ot{len(cs)}",
                            bufs=(OUT_BUFS if len(cs) == STORE_CHUNKS
                                  else TAIL_BUFS))
                        col0 = 0
                        total = 0
                        for c in cs:
                            if ops_mode == "noevac":
                                nt = emit_compute(g, c, u, None, 0)
                            else:
                                nt = emit_compute(g, c, u, ot, col0)
                            col0 += 3 * nt
                            total += nt
                            u += 1
                        if ops_mode == "noevac":
                            for c in cs:
                                nt = min(CHUNK, ROWS - CHUNK_STARTS[c])
                                dst = out_r[g][
                                    :, :, CHUNK_STARTS[c]:
                                    CHUNK_STARTS[c] + nt, :].rearrange(
                                    "a s t i -> (a s) t i")
                                nc.sync.dma_start(
                                    out=dst, in_=gt[:, : 3 * nt])
                        else:
                            emit_store(g, cs, ot, total)
    nc.compile()
    return nc


def _make_in_maps(x, y):
    x_flat = x.reshape(256, 3, 3)
    sr = np.arange(SEGS)
    in_maps = []
    for c in range(N_CORES):
        b = c // 2
        xg = x_flat[c * N_PER_CORE:(c + 1) * N_PER_CORE].reshape(
            N_GROUPS, N_SUB, 3, 3)  # [g, a, i, j]
        wall = np.zeros((SEGS, 3, N_GROUPS, 3, N_SUB, SEGS), np.float32)
        wall[sr, :, :, :, :, sr] = xg.transpose(3, 0, 2, 1)[None]
        xw_np = np.ascontiguousarray(wall.reshape(K, N_GROUPS * 3 * P))
        ys_np = np.zeros((K, ROWS_PAD), np.float32)
        ys_np[:, :ROWS] = (
            y[b].reshape(SEGS, ROWS, 3).transpose(0, 2, 1).reshape(K, ROWS))
        in_maps.append({"xw": xw_np, "ys": ys_np})
    return in_maps


def kernel(x: np.ndarray, y: np.ndarray) -> np.ndarray:
    global LAST, _CACHED_NC
    x = np.ascontiguousarray(x, dtype=np.float32)
    y = np.ascontiguousarray(y, dtype=np.float32)
    assert x.shape == (4, 64, 3, 3) and y.shape == (4, 100000, 3)

    if _CACHED_NC is None:
        _CACHED_NC = build_bass()
    nc = _CACHED_NC

    in_maps = _make_in_maps(x, y)
    res = run_bass_kernel_spmd(
        nc, in_maps, core_ids=list(range(N_CORES)), trace=TRACE,
    )
    LAST = res
    out = np.concatenate([r["out"] for r in res.results], axis=0)
    return out.reshape(4, 64, 100000, 3)


def _prepare_exec(nc, in_maps):
    """Build a jitted 8-core executor for `nc` with device-resident inputs.

    Returns (run_once, zeros) where run_once(outs) executes the NEFF once per
    core and returns new device outputs (pass them back in as the donated
    output buffers for the next call)."""
    import jax
    import concourse.mybir as mybir_
    from jax.experimental.shard_map import shard_map
    from jax.sharding import Mesh, NamedSharding, PartitionSpec
    from concourse.bass2jax import (
        _bass_exec_p, install_neuronx_cc_hook, partition_id_tensor,
    )

    install_neuronx_cc_hook()
    partition_name = nc.partition_id_tensor.name if nc.partition_id_tensor else None
    in_names, out_names, out_avals, zero_outs = [], [], [], []
    for alloc in nc.m.functions[0].allocations:
        if not isinstance(alloc, mybir_.MemoryLocationSet):
            continue
        name = alloc.memorylocations[0].name
        if alloc.kind == "ExternalInput":
            if name != partition_name:
                in_names.append(name)
        elif alloc.kind == "ExternalOutput":
            shape = tuple(alloc.tensor_shape)
            dtype = mybir_.dt.np(alloc.dtype)
            out_names.append(name)
            out_avals.append(jax.core.ShapedArray(shape, dtype))
            zero_outs.append(np.zeros(shape, dtype))
    n_params = len(in_names)
    n_outs = len(out_names)
    all_names = in_names + out_names + ([partition_name] if partition_name else [])

    def _body(*args):
        operands = list(args)
        if partition_name is not None:
            operands.append(partition_id_tensor())
        outs = _bass_exec_p.bind(
            *operands,
            out_avals=tuple(out_avals),
            in_names=tuple(all_names),
            out_names=tuple(out_names),
            lowering_input_output_aliases=(),
            sim_require_finite=True,
            sim_require_nnan=True,
            nc=nc,
        )
        return tuple(outs)

    devices = jax.devices()[:N_CORES]
    mesh = Mesh(np.asarray(devices), ("core",))
    spec = PartitionSpec("core")
    sharded = jax.jit(
        shard_map(
            _body, mesh=mesh, in_specs=(spec,) * (n_params + n_outs),
            out_specs=(spec,) * n_outs, check_rep=False,
        ),
        donate_argnums=tuple(range(n_params, n_params + n_outs)),
        keep_unused=True,
    )
    sh = NamedSharding(mesh, spec)
    ins_dev = [
        jax.device_put(
            np.concatenate([np.asarray(m[name]) for m in in_maps], axis=0), sh
        )
        for name in in_names
    ]
    zeros = [
        jax.device_put(
            np.zeros((N_CORES * z.shape[0], *z.shape[1:]), z.dtype), sh
        )
        for z in zero_outs
    ]

    def run_once(outs):
        res = sharded(*ins_dev, *outs)
        jax.block_until_ready(res)
        return list(res)

    return run_once, zeros


def bench(x, y, reps_pair=(9, 257), samples=24, ops_mode="full"):
    """Measure steady-state per-workload HW time by differencing kernels
    that run the workload `reps_pair[0]` vs `reps_pair[1]` times.

    Samples of the two kernels are interleaved so slow device-state drift
    (power/thermal) cancels in the per-pair difference; the reported value
    is the median of the per-pair differences."""
    import time
    x = np.ascontiguousarray(x, dtype=np.float32)
    y = np.ascontiguousarray(y, dtype=np.float32)
    in_maps = _make_in_maps(x, y)
    r1, r2 = reps_pair
    execs = {}
    for reps in reps_pair:
        nc = build_bass(reps=reps, ops_mode=ops_mode)
        run_once, zeros = _prepare_exec(nc, in_maps)
        outs = run_once(zeros)  # compile + warm
        execs[reps] = (run_once, outs)
    raw = {r1: [], r2: []}
    for _ in range(samples):
        for reps in reps_pair:
            run_once, outs = execs[reps]
            t0 = time.perf_counter()
            outs = run_once(outs)
            raw[reps].append(time.perf_counter() - t0)
            execs[reps] = (run_once, outs)
    med = {}
    for reps in reps_pair:
        ts = sorted(raw[reps])
        med[reps] = ts[len(ts) // 2]
        print(f"reps={reps}: med {med[reps]*1e3:.2f} ms  "
              f"all {[f'{t*1e3:.1f}' for t in ts]}")
    return (med[r2] - med[r1]) / (r2 - r1) * 1e9


# revision 26
# speedup vs baseline: 1.5832x; 1.5832x over previous
"""Trainium2 Bass kernel for nn_BatchMatMulModule.

Computes out = einsum("bnij,bmj->bnmi", x, y) with
  x: [4, 64, 3, 3] f32, y: [4, 100000, 3] f32 -> out: [4, 64, 100000, 3] f32.

The output (307 MB) dwarfs the inputs (4.8 MB), so the kernel is bound by
HBM write bandwidth (~358 GB/s per NeuronCore => ~107 us floor for the
38.4 MB each core stores). Strategy (v2, TensorE-based):

- Shard the 256 flat (b, n) pairs across 8 cores: core c handles b = c // 2
  and 32 consecutive n. Output slice [32, 100000, 3] is contiguous in DRAM.
- SBUF/PSUM layout: partition p = a * 32 + s with a = n % 4 (within a group
  of 4 n's) and s = m-segment (100000 = 32 segments x 3125 rows).
- The contraction out[(a,s), t, i] = sum_j x[n,i,j] * y[(s,t),j] is run on
  the idle TensorE as a matmul with a block-diagonal stationary operand:
    W_{g,i}[(s',j), (a,s)] = x[g*4+a, i, j] * delta(s,s')   [96 x 128]
    Y[(s',j), t] = y[s'*3125 + t, j]                        [96 x 3125]
    psum_i = W_{g,i}.T @ Y[:, t0:t1]                        [128 x <=512]
  Operands are fp32 bitcast to float32r (FP22-truncated): full PE rate at
  N >= 256, rel err ~1e-4 (harness gate is 2e-2).
- PSUM cannot be DMA'd, so the mandatory PSUM->SBUF evacuation doubles as
  the i-interleave: one ACT/DVE copy per chunk reads the 3 psum planes
  (strided) and writes the (t, i)-interleaved SBUF tile (contiguous), which
  then stores as a 768 KB contiguous-per-partition DMA.
- Engine budget per core: DMA-out ~107 us (bottleneck), PE ~25 us,
  ACT/DVE alternate evacuation chunks at ~40 us each.
"""

import numpy as np

import concourse.bacc as bacc
import concourse.mybir as mybir
from concourse.bass_utils import run_bass_kernel_spmd
from concourse.tile import TileContext

N_CORES = 8
P = 128
N_PER_CORE = 32   # n per core
N_SUB = 4         # a: n's packed per partition-group
N_GROUPS = N_PER_CORE // N_SUB  # 8 groups, one weight set each
SEGS = P // N_SUB               # 32 m-segments
M = 100000
ROWS = M // SEGS                # 3125 t-rows per segment
K = SEGS * 3                    # 96 contraction rows (s', j)

CHUNK = 512                     # t-chunk = one PSUM bank of fp32
CHUNK_STARTS = list(range(0, ROWS, CHUNK))
ROWS_PAD = ROWS + (ROWS % 2)    # fp32r matmul needs even free sizes; pad tail

TRACE = False
LAST = None  # last BassKernelResults, for test harness introspection

_CACHED_NC = None

# Tuning knobs.
EVAC_MODE = "fused"  # "fused": 1 copy/chunk (strided psum read, contig write)
                     # "plane3": 3 copies/chunk (contig read, strided write)
OUT_BUFS = 5
TAIL_BUFS = 5   # bufs for the merged tail store group's tiles
PSUM_BUFS = 2
Y_SPLITS = 4
STORE_CHUNKS = 3     # chunks coalesced per store DMA (trailing short group merged)
STORE_ENGINE = "sync"  # "sync" | "alt" (alternate sync/scalar HWDGE queues)


def build_bass(reps: int = 1, ops_mode: str = "full"):
    nc = bacc.Bacc(
        "TRN2",
        debug=False,
        enable_asserts=False,
        target_bir_lowering=False,
        num_devices=N_CORES,
    )
    f32 = mybir.dt.float32
    f32r = mybir.dt.float32r
    copy = mybir.ActivationFunctionType.Copy

    # Host-prearranged inputs (float32r: fp32 bytes, FP22-truncated by the PE):
    #  xw[k, (g*3+i)*128 + a*32 + s] = x[g*4+a, i, j] * delta(s, k//3), j=k%3
    #  ys[k, t] = y[b, (k//3)*3125 + t, k%3]
    xw = nc.dram_tensor("xw", [K, N_GROUPS * 3 * P], f32r, kind="ExternalInput").ap()
    ys = nc.dram_tensor("ys", [K, ROWS_PAD], f32r, kind="ExternalInput").ap()
    out = nc.dram_tensor("out", [N_PER_CORE, M, 3], f32, kind="ExternalOutput").ap()

    with TileContext(nc) as tc:
        with (
            tc.tile_pool(name="const", bufs=1) as cpool,
            tc.tile_pool(name="outp", bufs=OUT_BUFS) as opool,
            tc.tile_pool(name="psp", bufs=PSUM_BUFS, space="PSUM") as pspool,
        ):
            wt = cpool.tile([K, N_GROUPS * 3 * P], f32r)
            # First group's weights first so g=0 matmuls start early.
            nc.sync.dma_start(out=wt[:, : 3 * P], in_=xw[:, : 3 * P])
            nc.sync.dma_start(out=wt[:, 3 * P:], in_=xw[:, 3 * P:])

            y_tile = cpool.tile([K, ROWS_PAD], f32r)
            yb = [ROWS_PAD * h // Y_SPLITS for h in range(Y_SPLITS + 1)]
            for h in range(Y_SPLITS):
                nc.sync.dma_start(
                    out=y_tile[:, yb[h]:yb[h + 1]], in_=ys[:, yb[h]:yb[h + 1]]
                )

            out_r = out.rearrange("(g a) (s t) i -> g a s t i", a=N_SUB, s=SEGS)

            gt = None
            if ops_mode in ("store", "noevac"):
                gt = cpool.tile([P, 3 * CHUNK], f32)
                nc.vector.memset(gt[:], 0.0)

            def emit_compute(g, c, u, ot, col0):
                """Matmuls + evacuation for chunk (g, c); interleaved result
                lands at ot[:, col0 : col0 + 3*nt]. Returns nt."""
                t0 = CHUNK_STARTS[c]
                nt = min(CHUNK, ROWS - t0)           # rows actually stored
                nt_mm = nt + (nt % 2)                # fp32r: even matmul width
                pst = pspool.tile([P, 3 * CHUNK], f32, name="ps", tag="ps")
                for i in range(3):
                    blk = (g * 3 + i) * P
                    nc.tensor.matmul(
                        pst[:, i * CHUNK: i * CHUNK + nt_mm],
                        wt[:, blk: blk + P],
                        y_tile[:, t0: t0 + nt_mm],
                        start=True,
                        stop=True,
                    )
                if ops_mode == "noevac":
                    return nt
                # psum viewed [p, i, t] (planes at CHUNK spacing), out (t, i).
                psv = pst.rearrange("p (i t) -> p i t", i=3)[:, :, :nt]
                otv = ot[:, col0: col0 + 3 * nt]
                if EVAC_MODE == "fused":
                    src = psv.rearrange("p i t -> p t i")
                    dst_sb = otv.rearrange("p (t i) -> p t i", i=3)
                    if u % 2 == 0:
                        nc.scalar.activation(out=dst_sb, in_=src, func=copy)
                    else:
                        nc.vector.tensor_copy(out=dst_sb, in_=src)
                else:
                    ov = otv.rearrange("p (t i) -> p t i", i=3)
                    for i in range(3):
                        if (i + u) % 3 != 0:
                            nc.scalar.activation(
                                out=ov[:, :, i], in_=psv[:, i, :], func=copy)
                        else:
                            nc.vector.tensor_copy(
                                out=ov[:, :, i], in_=psv[:, i, :])
                return nt

            store_n = [0]

            def emit_store(g, cs, ot, total_nt):
                t0 = CHUNK_STARTS[cs[0]]
                dst = out_r[g][:, :, t0: t0 + total_nt, :].rearrange(
                    "a s t i -> (a s) t i")
                eng = nc.sync
                if STORE_ENGINE == "alt" and store_n[0] % 2 == 1:
                    eng = nc.scalar
                store_n[0] += 1
                eng.dma_start(out=dst, in_=ot[:, : 3 * total_nt])

            nch = len(CHUNK_STARTS)
            groups = [list(range(c, min(c + STORE_CHUNKS, nch)))
                      for c in range(0, nch, STORE_CHUNKS)]
            if len(groups) > 1 and len(groups[-1]) == 1:
                groups[-2].extend(groups.pop())

            u = 0
            for _ in range(reps):
                for g in range(N_GROUPS):
                    for cs in groups:
                        span = 3 * (CHUNK * (len(cs) - 1)
                                    + min(CHUNK, ROWS - CHUNK_STARTS[cs[-1]]))
                        if ops_mode == "store":
                            # store-only probe: reuse the const garbage tile
                            for c in cs:
                                nt = min(CHUNK, ROWS - CHUNK_STARTS[c])
                                dst = out_r[g][
                                    :, :, CHUNK_STARTS[c]:
                                    CHUNK_STARTS[c] + nt, :].rearrange(
                                    "a s t i -> (a s) t i")
                                nc.sync.dma_start(
                                    out=dst, in_=gt[:, : 3 * nt])
                            u += len(cs)
                            continue
                        ot = opool.tile(
                            [P, span], f32, name="ot",
                            tag=f"ot{len(cs)}",
                            bufs=(OUT_BUFS if len(cs) == STORE_CHUNKS
                                  else TAIL_BUFS))
                        col0 = 0
                        total = 0
                        for c in cs:
                            if ops_mode == "noevac":
                                nt = emit_compute(g, c, u, None, 0)
                            else:
                                nt = emit_compute(g, c, u, ot, col0)
                            col0 += 3 * nt
                            total += nt
                            u += 1
                        if ops_mode == "noevac":
                            for c in cs:
                                nt = min(CHUNK, ROWS - CHUNK_STARTS[c])
                                dst = out_r[g][
                                    :, :, CHUNK_STARTS[c]:
                                    CHUNK_STARTS[c] + nt, :].rearrange(
                                    "a s t i -> (a s) t i")
                                nc.sync.dma_start(
                                    out=dst, in_=gt[:, : 3 * nt])
                        else:
                            emit_store(g, cs, ot, total)
    nc.compile()
    return nc


def _make_in_maps(x, y):
    x_flat = x.reshape(256, 3, 3)
    sr = np.arange(SEGS)
    in_maps = []
    for c in range(N_CORES):
        b = c // 2
        xg = x_flat[c * N_PER_CORE:(c + 1) * N_PER_CORE].reshape(
            N_GROUPS, N_SUB, 3, 3)  # [g, a, i, j]
        wall = np.zeros((SEGS, 3, N_GROUPS, 3, N_SUB, SEGS), np.float32)
        wall[sr, :, :, :, :, sr] = xg.transpose(3, 0, 2, 1)[None]
        xw_np = np.ascontiguousarray(wall.reshape(K, N_GROUPS * 3 * P))
        ys_np = np.zeros((K, ROWS_PAD), np.float32)
        ys_np[:, :ROWS] = (
            y[b].reshape(SEGS, ROWS, 3).transpose(0, 2, 1).reshape(K, ROWS))
        in_maps.append({"xw": xw_np, "ys": ys_np})
    return in_maps


def kernel(x: np.ndarray, y: np.ndarray) -> np.ndarray:
    global LAST, _CACHED_NC
    x = np.ascontiguousarray(x, dtype=np.float32)
    y = np.ascontiguousarray(y, dtype=np.float32)
    assert x.shape == (4, 64, 3, 3) and y.shape == (4, 100000, 3)

    if _CACHED_NC is None:
        _CACHED_NC = build_bass()
    nc = _CACHED_NC

    in_maps = _make_in_maps(x, y)
    res = run_bass_kernel_spmd(
        nc, in_maps, core_ids=list(range(N_CORES)), trace=TRACE,
    )
    LAST = res
    out = np.concatenate([r["out"] for r in res.results], axis=0)
    return out.reshape(4, 64, 100000, 3)


def _prepare_exec(nc, in_maps):
    """Build a jitted 8-core executor for `nc` with device-resident inputs.

    Returns (run_once, zeros) where run_once(outs) executes the NEFF once per
    core and returns new device outputs (pass them back in as the donated
    output buffers for the next call)."""
    import jax
    import concourse.mybir as mybir_
    from jax.experimental.shard_map import shard_map
    from jax.sharding import Mesh, NamedSharding, PartitionSpec
    from concourse.bass2jax import (
        _bass_exec_p, install_neuronx_cc_hook, partition_id_tensor,
    )

    install_neuronx_cc_hook()
    partition_name = nc.partition_id_tensor.name if nc.partition_id_tensor else None
    in_names, out_names, out_avals, zero_outs = [], [], [], []
    for alloc in nc.m.functions[0].allocations:
        if not isinstance(alloc, mybir_.MemoryLocationSet):
            continue
        name = alloc.memorylocations[0].name
        if alloc.kind == "ExternalInput":
            if name != partition_name:
                in_names.append(name)
        elif alloc.kind == "ExternalOutput":
            shape = tuple(alloc.tensor_shape)
            dtype = mybir_.dt.np(alloc.dtype)
            out_names.append(name)
            out_avals.append(jax.core.ShapedArray(shape, dtype))
            zero_outs.append(np.zeros(shape, dtype))
    n_params = len(in_names)
    n_outs = len(out_names)
    all_names = in_names + out_names + ([partition_name] if partition_name else [])

    def _body(*args):
        operands = list(args)
        if partition_name is not None:
            operands.append(partition_id_tensor())
        outs = _bass_exec_p.bind(
            *operands,
            out_avals=tuple(out_avals),
            in_names=tuple(all_names),
            out_names=tuple(out_names),
            lowering_input_output_aliases=(),
            sim_require_finite=True,
            sim_require_nnan=True,
            nc=nc,
        )
        return tuple(outs)

    devices = jax.devices()[:N_CORES]
    mesh = Mesh(np.asarray(devices), ("core",))
    spec = PartitionSpec("core")
    sharded = jax.jit(
        shard_map(
            _body, mesh=mesh, in_specs=(spec,) * (n_params + n_outs),
            out_specs=(spec,) * n_outs, check_rep=False,
        ),
        donate_argnums=tuple(range(n_params, n_params + n_outs)),
        keep_unused=True,
    )
    sh = NamedSharding(mesh, spec)
    ins_dev = [
        jax.device_put(
            np.concatenate([np.asarray(m[name]) for m in in_maps], axis=0), sh
        )
        for name in in_names
    ]
    zeros = [
        jax.device_put(
            np.zeros((N_CORES * z.shape[0], *z.shape[1:]), z.dtype), sh
        )
        for z in zero_outs
    ]

    def run_once(outs):
        res = sharded(*ins_dev, *outs)
        jax.block_until_ready(res)
        return list(res)

    return run_once, zeros


def bench(x, y, reps_pair=(9, 65), samples=24, ops_mode="full"):
    """Measure steady-state per-workload HW time by differencing kernels
    that run the workload `reps_pair[0]` vs `reps_pair[1]` times.

    Samples of the two kernels are interleaved so slow device-state drift
    (power/thermal) cancels in the per-pair difference; the reported value
    is the median of the per-pair differences."""
    import time
    x = np.ascontiguousarray(x, dtype=np.float32)
    y = np.ascontiguousarray(y, dtype=np.float32)
    in_maps = _make_in_maps(x, y)
    r1, r2 = reps_pair
    execs = {}
    for reps in reps_pair:
        nc = build_bass(reps=reps, ops_mode=ops_mode)
        run_once, zeros = _prepare_exec(nc, in_maps)
        outs = run_once(zeros)  # compile + warm
        execs[reps] = (run_once, outs)
    raw = {r1: [], r2: []}
    for _ in range(samples):
        for reps in reps_pair:
            run_once, outs = execs[reps]
            t0 = time.perf_counter()
            outs = run_once(outs)
            raw[reps].append(time.perf_counter() - t0)
            execs[reps] = (run_once, outs)
    med = {}
    for reps in reps_pair:
        ts = sorted(raw[reps])
        med[reps] = ts[len(ts) // 2]
        print(f"reps={reps}: med {med[reps]*1e3:.2f} ms  "
              f"all {[f'{t*1e3:.1f}' for t in ts]}")
    return (med[r2] - med[r1]) / (r2 - r1) * 1e9
